# revision 2
# baseline (speedup 1.0000x reference)
"""Trainium2 Bass kernel for ContextAwareArtRecSys (gnn_message_passing).

Math fold: with vu = wo[:, :128] @ Wu, vi = wo[:, 128:] @ Wi,
c = wo[:, :128]@bu + wo[:, 128:]@bi + bo:
    score[e] = (z_u @ vu)[src] + (z_i @ vi)[dst] + c.

Device plan (SPMD over 8 cores):
  * matvec: z shards stream in block-per-partition layout (big DMA
    descriptors), fused multiply+reduce on DVE (scalar_tensor_tensor).
  * ONE AllGather moves the concatenated (user ++ item) score shard of
    every core (18816 f32 per rank).
  * per-edge lookups run on two engines in parallel:
      - user lookups for the "A-range" edge slots: GPSIMD ap_gather from
        an SBUF-staged 25088-entry user slice (partition 16g holds the
        src-half g//4), 8 Q7 cores pipelining 4-index read bursts;
      - everything else (item lookups for A-range, user+item for the
        B-range) via SWDGE indirect DMA from the allgathered table.
  * DVE adds join the legs; HWDGE stores emit per-core output blocks.
Host does only layout: slicing/padding z, binning edge slots by src
half, index localization, inverse permutation of the output.
"""

import numpy as np

N_CORES = 8
N_USERS, N_ITEMS, E, H = 50000, 100000, 500000, 256
HALF = H // 2

U_SH = N_USERS // N_CORES          # 6250
I_SH = N_ITEMS // N_CORES          # 12500
UB = 49                            # user rows per partition
IB = 98                            # item rows per partition
U_PAD = 128 * UB                   # 6272
I_PAD = 128 * IB                   # 12544
C_PAD = U_PAD + I_PAD              # 18816 = per-core concat shard
SLICE = 4 * U_PAD                  # 25088 staged user-slice entries

E_SH = E // N_CORES                # 62500 edges per core
NIG = 8192                         # slots per (core, stream)
CAP = 8 * NIG
AQG = 5632                         # A-range (ap_gather) slots per stream
NSG = NIG - AQG                    # 2560 B-range slots per stream
GCH = 704                          # idx per ap_gather instruction (8/stream)
NB = 8 * NSG                       # 20480 B slots per core
COLS_A = AQG // 128                # 44 idx cols per item-A indirect instr
NBI = 8                            # B indirect instructions per leg
BCH = NB // NBI                    # 2560 elements per B instruction
COLS_B = BCH // 128                # 20 idx cols per B indirect instr

_CACHE = {}


def _build():
    if "nc" in _CACHE:
        return _CACHE["nc"]
    import concourse.bass as bass
    import concourse.tile as tile
    import concourse.mybir as mybir
    from concourse import bacc, library_config
    from concourse.bass import IndirectOffsetOnAxis

    f32 = mybir.dt.float32
    i16 = mybir.dt.int16
    i32 = mybir.dt.int32

    nc = bacc.Bacc("TRN2", target_bir_lowering=False, debug=False,
                   num_devices=N_CORES, num_swdge_queues=4)

    zu = nc.dram_tensor("zu", [128, UB * H], f32, kind="ExternalInput")
    zi = nc.dram_tensor("zi", [128, IB * H], f32, kind="ExternalInput")
    w_user = nc.dram_tensor("w_user", [HALF, H], f32, kind="ExternalInput")
    w_item = nc.dram_tensor("w_item", [HALF, H], f32, kind="ExternalInput")
    wo_u = nc.dram_tensor("wo_u", [HALF, 1], f32, kind="ExternalInput")
    wo_i = nc.dram_tensor("wo_i", [HALF, 1], f32, kind="ExternalInput")
    b_user = nc.dram_tensor("b_user", [HALF, 1], f32, kind="ExternalInput")
    b_item = nc.dram_tensor("b_item", [HALF, 1], f32, kind="ExternalInput")
    b_out = nc.dram_tensor("b_out", [1, 1], f32, kind="ExternalInput")

    idxau = nc.dram_tensor("idxau", [128, 8 * (GCH // 16)], i16,
                           kind="ExternalInput")
    idxda = nc.dram_tensor("idxda", [128, 8 * COLS_A], i32,
                           kind="ExternalInput")
    idxdu = nc.dram_tensor("idxdu", [128, NBI * COLS_B], i32,
                           kind="ExternalInput")
    idxdi = nc.dram_tensor("idxdi", [128, NBI * COLS_B], i32,
                           kind="ExternalInput")
    outA = nc.dram_tensor("outA", [8, AQG], f32, kind="ExternalOutput")
    outB = nc.dram_tensor("outB", [NBI, BCH], f32, kind="ExternalOutput")

    s_cu = nc.dram_tensor("s_cu", [U_PAD, 1], f32)
    s_ci = nc.dram_tensor("s_ci", [I_PAD, 1], f32)
    s_uf = nc.dram_tensor("s_uf", [N_CORES * U_PAD, 1], f32,
                          addr_space="Shared")
    s_if = nc.dram_tensor("s_if", [N_CORES * I_PAD, 1], f32,
                          addr_space="Shared")
    s_ufl = nc.dram_tensor("s_ufl", [N_CORES * U_PAD, 1], f32)
    s_ifl = nc.dram_tensor("s_ifl", [N_CORES * I_PAD, 1], f32)

    groups = [list(range(N_CORES))]

    with tile.TileContext(nc) as tc:
        with (
            tc.tile_pool(name="consts", bufs=1) as consts,
            tc.tile_pool(name="spool", bufs=1) as spool,
            tc.tile_pool(name="psum", bufs=2, space="PSUM") as psum,
        ):
            nc.gpsimd.load_library(library_config.ap_gather)

            idxau_t = spool.tile([128, 8 * (GCH // 16)], i16)
            nc.scalar.dma_start(idxau_t[:], idxau.ap())
            idxda_t = spool.tile([128, 8 * COLS_A], i32)
            nc.scalar.dma_start(idxda_t[:], idxda.ap())
            idxdu_t = spool.tile([128, NBI * COLS_B], i32)
            nc.scalar.dma_start(idxdu_t[:], idxdu.ap())
            idxdi_t = spool.tile([128, NBI * COLS_B], i32)
            nc.scalar.dma_start(idxdi_t[:], idxdi.ap())

            # ---- fold vu / vi / c on PE ----
            wu_t = consts.tile([HALF, H], f32)
            nc.sync.dma_start(wu_t[:], w_user.ap())
            wi_t = consts.tile([HALF, H], f32)
            nc.sync.dma_start(wi_t[:], w_item.ap())
            wou_t = consts.tile([HALF, 1], f32)
            nc.sync.dma_start(wou_t[:], wo_u.ap())
            woi_t = consts.tile([HALF, 1], f32)
            nc.sync.dma_start(woi_t[:], wo_i.ap())
            bu_t = consts.tile([HALF, 1], f32)
            nc.sync.dma_start(bu_t[:], b_user.ap())
            bi_t = consts.tile([HALF, 1], f32)
            nc.sync.dma_start(bi_t[:], b_item.ap())
            bo_t = consts.tile([1, 1], f32)
            nc.sync.dma_start(bo_t[:], b_out.ap())

            ones_kk = consts.tile([HALF, HALF], f32)
            nc.vector.memset(ones_kk[:], 1.0)
            wou_rep = consts.tile([HALF, HALF], f32)
            nc.vector.tensor_scalar_mul(wou_rep[:], ones_kk[:], wou_t[:])
            woi_rep = consts.tile([HALF, HALF], f32)
            nc.vector.tensor_scalar_mul(woi_rep[:], ones_kk[:], woi_t[:])

            vu_ps = psum.tile([HALF, H], f32, tag="vps")
            nc.tensor.matmul(vu_ps[:], wou_rep[:], wu_t[:], start=True, stop=True)
            vu_t = consts.tile([HALF, H], f32)
            nc.vector.tensor_copy(vu_t[:], vu_ps[:])
            vi_ps = psum.tile([HALF, H], f32, tag="vps")
            nc.tensor.matmul(vi_ps[:], woi_rep[:], wi_t[:], start=True, stop=True)
            vi_t = consts.tile([HALF, H], f32)
            nc.vector.tensor_copy(vi_t[:], vi_ps[:])

            ones_k1 = consts.tile([HALF, 128], f32)
            nc.vector.memset(ones_k1[:], 1.0)
            cu_ps = psum.tile([128, 1], f32, tag="cps")
            bub = consts.tile([HALF, 128], f32)
            nc.vector.tensor_scalar_mul(bub[:], ones_k1[:], bu_t[:])
            bib = consts.tile([HALF, 128], f32)
            nc.vector.tensor_scalar_mul(bib[:], ones_k1[:], bi_t[:])
            nc.tensor.matmul(cu_ps[:], bub[:], wou_t[:], start=True, stop=False)
            nc.tensor.matmul(cu_ps[:], bib[:], woi_t[:], start=False, stop=False)
            nc.tensor.matmul(
                cu_ps[:], ones_k1[0:1, :], bo_t[:], start=False, stop=True
            )
            c_t = consts.tile([128, 1], f32)
            nc.vector.tensor_copy(c_t[:], cu_ps[:])

            # ---- phase 1: score shards via fused DVE matvec ----
            prod_t = consts.tile([128, 1], f32)

            def mv_tile(zt, j, v_t, acc_col):
                nc.vector.scalar_tensor_tensor(
                    out=prod_t[:].broadcast_to((128, H)),
                    in0=zt[:, j * H:(j + 1) * H],
                    scalar=1.0,
                    in1=v_t[:],
                    op0=mybir.AluOpType.mult,
                    op1=mybir.AluOpType.mult,
                    accum_out=acc_col,
                )

            su_sb = spool.tile([128, UB], f32)
            si_sb = spool.tile([128, IB], f32)

            with tc.tile_pool(name="zpool", bufs=2) as zpool:
                off = 0
                for ch in (25, 24):
                    zt = zpool.tile([128, ch * H], f32, tag="z")
                    nc.sync.dma_start(zt[:], zu.ap()[:, off * H:(off + ch) * H])
                    for j in range(ch):
                        mv_tile(zt, j, vu_t, su_sb[:, off + j:off + j + 1])
                    off += ch
                nc.vector.tensor_scalar_add(su_sb[:], su_sb[:], c_t[:])
                nc.sync.dma_start(
                    s_cu.ap().rearrange("(p j) one -> p (j one)", p=128),
                    su_sb[:],
                )
                nc.gpsimd.collective_compute(
                    "AllGather", mybir.AluOpType.bypass,
                    replica_groups=groups, ins=[s_cu.ap()], outs=[s_uf.ap()],
                )
                nc.scalar.dma_start(s_ufl.ap(), s_uf.ap())
                off = 0
                for ch in (25, 25, 24, 24):
                    zt = zpool.tile([128, ch * H], f32, tag="z")
                    nc.sync.dma_start(zt[:], zi.ap()[:, off * H:(off + ch) * H])
                    for j in range(ch):
                        mv_tile(zt, j, vi_t, si_sb[:, off + j:off + j + 1])
                    off += ch
                nc.sync.dma_start(
                    s_ci.ap().rearrange("(p j) one -> p (j one)", p=128),
                    si_sb[:],
                )
                nc.gpsimd.collective_compute(
                    "AllGather", mybir.AluOpType.bypass,
                    replica_groups=groups, ins=[s_ci.ap()], outs=[s_if.ap()],
                )
                nc.scalar.dma_start(s_ifl.ap(), s_if.ap())

            # ---- phase 2: gathers on both engines ----
            with tc.tile_pool(name="gpool", bufs=1) as gpool:
                table_t = gpool.tile([128, SLICE], f32)
                # user slice g//4 is contiguous in the allgathered table
                s_ufl_r = s_ufl.ap().rearrange("(s x) one -> s (x one)", s=2)
                for g in range(8):
                    nc.scalar.dma_start(
                        table_t[16 * g:16 * g + 1, :],
                        s_ufl_r[g // 4:g // 4 + 1, :],
                    )

                gu_t = gpool.tile([128, AQG], f32)
                gi2a = gpool.tile([8, AQG], f32)
                gu2b = gpool.tile([NBI, BCH], f32)
                gi2b = gpool.tile([NBI, BCH], f32)

                _q = [0]

                def ind(out_tile, row, table_ap, idx_t, col0, ncols):
                    inst = nc.gpsimd.indirect_dma_start(
                        out=out_tile[row:row + 1, :].rearrange(
                            "one (c x) -> one c x", x=1),
                        out_offset=None,
                        in_=table_ap,
                        in_offset=IndirectOffsetOnAxis(
                            ap=idx_t[:, col0:col0 + ncols], axis=0
                        ),
                    )
                    qn = _q[0] % 4
                    _q[0] += 1
                    if qn:
                        inst.ins.queue = f"qPoolDynamic{qn}"
                    return inst

                def apg(ci):
                    nc.gpsimd.ap_gather(
                        out_ap=gu_t[:, ci * GCH:(ci + 1) * GCH],
                        in_ap=table_t[:],
                        idxs_ap=idxau_t[:, ci * (GCH // 16):(ci + 1) * (GCH // 16)],
                        channels=128,
                        num_elems=SLICE,
                        d=1,
                        num_idxs=GCH,
                    )

                # user-side legs first (user AG lands early), item after
                for m in range(NBI):
                    ind(gu2b, m, s_ufl.ap(), idxdu_t, m * COLS_B, COLS_B)
                # alternate: user ap chunks with item-A indirects
                for x in range(8):
                    ind(gi2a, x, s_ifl.ap(), idxda_t, x * COLS_A, COLS_A)
                    apg(x)
                for m in range(NBI):
                    ind(gi2b, m, s_ifl.ap(), idxdi_t, m * COLS_B, COLS_B)
                # outB join
                nc.vector.tensor_add(gu2b[:], gu2b[:], gi2b[:])
                nc.sync.dma_start(outB.ap(), gu2b[:])

                # outA join: realign user ap values, add item, store
                ga_u = gpool.tile([8, AQG], f32)
                nc.scalar.dma_start(ga_u[:], gu_t[0:128:16, :])
                nc.vector.tensor_add(ga_u[:], ga_u[:], gi2a[:])
                nc.sync.dma_start(outA.ap(), ga_u[:])

    nc.compile()
    _CACHE["nc"] = nc
    return nc


def _wrap_dma_idx(flat, n_instr, cols):
    """Spray order: idx[p, q*cols + c] = flat[q*(cols*128) + c*128 + p]."""
    seg = flat.reshape(n_instr, cols, 128)
    return np.ascontiguousarray(
        seg.transpose(2, 0, 1).reshape(128, n_instr * cols))


def _wrap_ap_idx(flat):
    """flat [8 streams, AQG] -> [128, 8*GCH/16] int16 with slot
    t = c*GCH + s*16 + p at idx[16g+p, c*(GCH//16)+s]."""
    A = flat.reshape(8, 8, GCH // 16, 16)          # [g, c, s, p]
    return np.ascontiguousarray(
        A.transpose(0, 3, 1, 2).reshape(128, 8 * (GCH // 16)))


def _make_in_maps(inputs):
    z_user = np.ascontiguousarray(np.asarray(inputs["z_user"], dtype=np.float32))
    z_item = np.ascontiguousarray(np.asarray(inputs["z_item"], dtype=np.float32))
    src = np.asarray(inputs["edge_src"]).astype(np.int64)
    dst = np.asarray(inputs["edge_dst"]).astype(np.int64)
    w_user = np.asarray(inputs["w_user"], dtype=np.float32)
    w_item = np.asarray(inputs["w_item"], dtype=np.float32)
    b_user = np.asarray(inputs["b_user"], dtype=np.float32).reshape(HALF, 1)
    b_item = np.asarray(inputs["b_item"], dtype=np.float32).reshape(HALF, 1)
    w_out = np.asarray(inputs["w_out"], dtype=np.float32)
    b_out = np.asarray(inputs["b_out"], dtype=np.float32).reshape(1, 1)
    wo_u = w_out[0, :HALF].reshape(HALF, 1).copy()
    wo_i = w_out[0, HALF:].reshape(HALF, 1).copy()

    # global offsets into the allgathered tables
    gu32 = ((src // U_SH) * U_PAD + (src % U_SH)).astype(np.int64)
    gi32 = ((dst // I_SH) * I_PAD + (dst % I_SH)).astype(np.int64)
    # user slice-local offsets (ap leg)
    lu16 = (((src // U_SH) % 4) * U_PAD + (src % U_SH)).astype(np.int64)
    subin = (src // 25000).astype(np.int64)        # 0/1 src half

    in_maps = []
    placements = []
    for k in range(N_CORES):
        lo, hi = k * E_SH, (k + 1) * E_SH
        sb = subin[lo:hi]
        order = np.argsort(sb, kind="stable")
        n0 = int(np.sum(sb == 0))
        sel = order + lo                            # edges, su-sorted

        lu_ap = np.zeros((8, AQG), dtype=np.int64)   # ap leg user idx
        gi_ap = np.zeros((8, AQG), dtype=np.int64)   # item idx for A-range
        gu_b = np.zeros(NB, dtype=np.int64)
        gi_b = np.zeros(NB, dtype=np.int64)
        slots = np.empty(E_SH, dtype=np.int64)       # unified slot id

        bpos = 0
        for b, (s0, s1) in enumerate(((0, n0), (n0, E_SH))):
            eb = sel[s0:s1]
            nb_ = len(eb)
            na = min(nb_, 4 * AQG)
            assert nb_ - na <= NB - bpos, f"B overflow core {k}"
            ea = eb[:na]
            g = b * 4 + np.arange(na) // AQG
            t = np.arange(na) % AQG
            lu_ap[g, t] = lu16[ea]
            gi_ap[g, t] = gi32[ea]
            slots[s0:s0 + na] = g * AQG + t
            ebb = eb[na:]
            j = bpos + np.arange(len(ebb))
            gu_b[j] = gu32[ebb]
            gi_b[j] = gi32[ebb]
            slots[s0 + na:s1] = 8 * AQG + j
            bpos += len(ebb)

        idxau_k = _wrap_ap_idx(lu_ap).astype(np.int16)
        idxda_k = _wrap_dma_idx(gi_ap.reshape(-1), 8, COLS_A).astype(np.int32)
        idxdu_k = _wrap_dma_idx(gu_b, NBI, COLS_B).astype(np.int32)
        idxdi_k = _wrap_dma_idx(gi_b, NBI, COLS_B).astype(np.int32)

        zu_k = np.zeros((U_PAD, H), dtype=np.float32)
        zu_k[:U_SH] = z_user[k * U_SH:(k + 1) * U_SH]
        zi_k = np.zeros((I_PAD, H), dtype=np.float32)
        zi_k[:I_SH] = z_item[k * I_SH:(k + 1) * I_SH]

        placements.append((sel, slots))
        in_maps.append({
            "zu": zu_k.reshape(128, UB * H),
            "zi": zi_k.reshape(128, IB * H),
            "w_user": w_user,
            "w_item": w_item,
            "wo_u": wo_u,
            "wo_i": wo_i,
            "b_user": b_user,
            "b_item": b_item,
            "b_out": b_out,
            "idxau": idxau_k,
            "idxda": idxda_k,
            "idxdu": idxdu_k,
            "idxdi": idxdi_k,
        })
    return in_maps, placements


def _run(inputs, trace=False):
    from concourse.bass_utils import run_bass_kernel_spmd

    nc = _build()
    in_maps, placements = _make_in_maps(inputs)
    res = run_bass_kernel_spmd(
        nc, in_maps, core_ids=list(range(N_CORES)), trace=trace
    )
    full = np.empty(E, dtype=np.float32)
    for k in range(N_CORES):
        outa = np.asarray(res.results[k]["outA"]).reshape(-1)
        outb = np.asarray(res.results[k]["outB"]).reshape(-1)
        vals = np.concatenate([outa, outb])
        sel, slots = placements[k]
        full[sel] = vals[slots]
    return full.reshape(E, 1), res


def kernel(**inputs):
    full, _ = _run(inputs, trace=False)
    return full


# revision 3
# speedup vs baseline: 1.0016x; 1.0016x over previous
"""Trainium2 Bass kernel for ContextAwareArtRecSys (gnn_message_passing).

Math fold: with vu = wo[:, :128] @ Wu, vi = wo[:, 128:] @ Wi,
c = wo[:, :128]@bu + wo[:, 128:]@bi + bo:
    score[e] = (z_u @ vu)[src] + (z_i @ vi)[dst] + c.

Device plan (SPMD over 8 cores):
  * matvec: z shards stream in block-per-partition layout (big DMA
    descriptors), fused multiply+reduce on DVE (scalar_tensor_tensor).
  * ONE AllGather moves the concatenated (user ++ item) score shard of
    every core (18816 f32 per rank).
  * per-edge lookups run on two engines in parallel:
      - user lookups for the "A-range" edge slots: GPSIMD ap_gather from
        an SBUF-staged 25088-entry user slice (partition 16g holds the
        src-half g//4), 8 Q7 cores pipelining 4-index read bursts;
      - everything else (item lookups for A-range, user+item for the
        B-range) via SWDGE indirect DMA from the allgathered table.
  * DVE adds join the legs; HWDGE stores emit per-core output blocks.
Host does only layout: slicing/padding z, binning edge slots by src
half, index localization, inverse permutation of the output.
"""

import numpy as np

N_CORES = 8
N_USERS, N_ITEMS, E, H = 50000, 100000, 500000, 256
HALF = H // 2

U_SH = N_USERS // N_CORES          # 6250
I_SH = N_ITEMS // N_CORES          # 12500
UB = 49                            # user rows per partition
IB = 98                            # item rows per partition
U_PAD = 128 * UB                   # 6272
I_PAD = 128 * IB                   # 12544
C_PAD = U_PAD + I_PAD              # 18816 = per-core concat shard
SLICE = 4 * U_PAD                  # 25088 staged user-slice entries

E_SH = E // N_CORES                # 62500 edges per core
NIG = 8192                         # slots per (core, stream)
CAP = 8 * NIG
AQG = 5632                         # A-range (ap_gather) slots per stream
NSG = NIG - AQG                    # 2560 B-range slots per stream
GCH = 704                          # idx per ap_gather instruction (8/stream)
NB = 8 * NSG                       # 20480 B slots per core
COLS_A = AQG // 128                # 44 idx cols per item-A indirect instr
NBI = 8                            # B indirect instructions per leg
BCH = NB // NBI                    # 2560 elements per B instruction
COLS_B = BCH // 128                # 20 idx cols per B indirect instr

_CACHE = {}


def _build():
    if "nc" in _CACHE:
        return _CACHE["nc"]
    import concourse.bass as bass
    import concourse.tile as tile
    import concourse.mybir as mybir
    from concourse import bacc, library_config
    from concourse.bass import IndirectOffsetOnAxis

    f32 = mybir.dt.float32
    i16 = mybir.dt.int16
    i32 = mybir.dt.int32

    nc = bacc.Bacc("TRN2", target_bir_lowering=False, debug=False,
                   num_devices=N_CORES, num_swdge_queues=4)

    zu = nc.dram_tensor("zu", [128, UB * H], f32, kind="ExternalInput")
    zi = nc.dram_tensor("zi", [128, IB * H], f32, kind="ExternalInput")
    w_user = nc.dram_tensor("w_user", [HALF, H], f32, kind="ExternalInput")
    w_item = nc.dram_tensor("w_item", [HALF, H], f32, kind="ExternalInput")
    wo_u = nc.dram_tensor("wo_u", [HALF, 1], f32, kind="ExternalInput")
    wo_i = nc.dram_tensor("wo_i", [HALF, 1], f32, kind="ExternalInput")
    b_user = nc.dram_tensor("b_user", [HALF, 1], f32, kind="ExternalInput")
    b_item = nc.dram_tensor("b_item", [HALF, 1], f32, kind="ExternalInput")
    b_out = nc.dram_tensor("b_out", [1, 1], f32, kind="ExternalInput")

    idxau = nc.dram_tensor("idxau", [128, 8 * (GCH // 16)], i16,
                           kind="ExternalInput")
    idxda = nc.dram_tensor("idxda", [128, 8 * COLS_A], i32,
                           kind="ExternalInput")
    idxdu = nc.dram_tensor("idxdu", [128, NBI * COLS_B], i32,
                           kind="ExternalInput")
    idxdi = nc.dram_tensor("idxdi", [128, NBI * COLS_B], i32,
                           kind="ExternalInput")
    outA = nc.dram_tensor("outA", [8, AQG], f32, kind="ExternalOutput")
    outB = nc.dram_tensor("outB", [NBI, BCH], f32, kind="ExternalOutput")

    s_cu = nc.dram_tensor("s_cu", [U_PAD, 1], f32)
    s_ci = nc.dram_tensor("s_ci", [I_PAD, 1], f32)
    s_uf = nc.dram_tensor("s_uf", [N_CORES * U_PAD, 1], f32,
                          addr_space="Shared")
    s_if = nc.dram_tensor("s_if", [N_CORES * I_PAD, 1], f32,
                          addr_space="Shared")
    s_ufl = nc.dram_tensor("s_ufl", [N_CORES * U_PAD, 1], f32)
    s_ifl = nc.dram_tensor("s_ifl", [N_CORES * I_PAD, 1], f32)

    groups = [list(range(N_CORES))]

    with tile.TileContext(nc) as tc:
        with (
            tc.tile_pool(name="consts", bufs=1) as consts,
            tc.tile_pool(name="spool", bufs=1) as spool,
            tc.tile_pool(name="psum", bufs=2, space="PSUM") as psum,
        ):
            nc.gpsimd.load_library(library_config.ap_gather)

            idxau_t = spool.tile([128, 8 * (GCH // 16)], i16)
            nc.scalar.dma_start(idxau_t[:], idxau.ap())
            idxda_t = spool.tile([128, 8 * COLS_A], i32)
            nc.scalar.dma_start(idxda_t[:], idxda.ap())
            idxdu_t = spool.tile([128, NBI * COLS_B], i32)
            nc.scalar.dma_start(idxdu_t[:], idxdu.ap())
            idxdi_t = spool.tile([128, NBI * COLS_B], i32)
            nc.scalar.dma_start(idxdi_t[:], idxdi.ap())

            # ---- fold vu / vi / c on PE ----
            wu_t = consts.tile([HALF, H], f32)
            nc.sync.dma_start(wu_t[:], w_user.ap())
            wi_t = consts.tile([HALF, H], f32)
            nc.sync.dma_start(wi_t[:], w_item.ap())
            wou_t = consts.tile([HALF, 1], f32)
            nc.sync.dma_start(wou_t[:], wo_u.ap())
            woi_t = consts.tile([HALF, 1], f32)
            nc.sync.dma_start(woi_t[:], wo_i.ap())
            bu_t = consts.tile([HALF, 1], f32)
            nc.sync.dma_start(bu_t[:], b_user.ap())
            bi_t = consts.tile([HALF, 1], f32)
            nc.sync.dma_start(bi_t[:], b_item.ap())
            bo_t = consts.tile([1, 1], f32)
            nc.sync.dma_start(bo_t[:], b_out.ap())

            ones_kk = consts.tile([HALF, HALF], f32)
            nc.vector.memset(ones_kk[:], 1.0)
            wou_rep = consts.tile([HALF, HALF], f32)
            nc.vector.tensor_scalar_mul(wou_rep[:], ones_kk[:], wou_t[:])
            woi_rep = consts.tile([HALF, HALF], f32)
            nc.vector.tensor_scalar_mul(woi_rep[:], ones_kk[:], woi_t[:])

            vu_ps = psum.tile([HALF, H], f32, tag="vps")
            nc.tensor.matmul(vu_ps[:], wou_rep[:], wu_t[:], start=True, stop=True)
            vu_t = consts.tile([HALF, H], f32)
            nc.vector.tensor_copy(vu_t[:], vu_ps[:])
            vi_ps = psum.tile([HALF, H], f32, tag="vps")
            nc.tensor.matmul(vi_ps[:], woi_rep[:], wi_t[:], start=True, stop=True)
            vi_t = consts.tile([HALF, H], f32)
            nc.vector.tensor_copy(vi_t[:], vi_ps[:])

            ones_k1 = consts.tile([HALF, 128], f32)
            nc.vector.memset(ones_k1[:], 1.0)
            cu_ps = psum.tile([128, 1], f32, tag="cps")
            bub = consts.tile([HALF, 128], f32)
            nc.vector.tensor_scalar_mul(bub[:], ones_k1[:], bu_t[:])
            bib = consts.tile([HALF, 128], f32)
            nc.vector.tensor_scalar_mul(bib[:], ones_k1[:], bi_t[:])
            nc.tensor.matmul(cu_ps[:], bub[:], wou_t[:], start=True, stop=False)
            nc.tensor.matmul(cu_ps[:], bib[:], woi_t[:], start=False, stop=False)
            nc.tensor.matmul(
                cu_ps[:], ones_k1[0:1, :], bo_t[:], start=False, stop=True
            )
            c_t = consts.tile([128, 1], f32)
            nc.vector.tensor_copy(c_t[:], cu_ps[:])

            # ---- phase 1: score shards via fused DVE matvec ----
            prod_t = consts.tile([128, 1], f32)

            def mv_tile(zt, j, v_t, acc_col):
                nc.vector.scalar_tensor_tensor(
                    out=prod_t[:].broadcast_to((128, H)),
                    in0=zt[:, j * H:(j + 1) * H],
                    scalar=1.0,
                    in1=v_t[:],
                    op0=mybir.AluOpType.mult,
                    op1=mybir.AluOpType.mult,
                    accum_out=acc_col,
                )

            su_sb = spool.tile([128, UB], f32)
            si_sb = spool.tile([128, IB], f32)

            with tc.tile_pool(name="zpool", bufs=2) as zpool:
                off = 0
                for ch in (25, 24):
                    zt = zpool.tile([128, ch * H], f32, tag="z")
                    nc.sync.dma_start(zt[:], zu.ap()[:, off * H:(off + ch) * H])
                    for j in range(ch):
                        mv_tile(zt, j, vu_t, su_sb[:, off + j:off + j + 1])
                    off += ch
                nc.vector.tensor_scalar_add(su_sb[:], su_sb[:], c_t[:])
                nc.sync.dma_start(
                    s_cu.ap().rearrange("(p j) one -> p (j one)", p=128),
                    su_sb[:],
                )
                nc.gpsimd.collective_compute(
                    "AllGather", mybir.AluOpType.bypass,
                    replica_groups=groups, ins=[s_cu.ap()], outs=[s_uf.ap()],
                )
                nc.scalar.dma_start(s_ufl.ap(), s_uf.ap())
                off = 0
                for ch in (25, 25, 24, 24):
                    zt = zpool.tile([128, ch * H], f32, tag="z")
                    nc.sync.dma_start(zt[:], zi.ap()[:, off * H:(off + ch) * H])
                    for j in range(ch):
                        mv_tile(zt, j, vi_t, si_sb[:, off + j:off + j + 1])
                    off += ch
                nc.sync.dma_start(
                    s_ci.ap().rearrange("(p j) one -> p (j one)", p=128),
                    si_sb[:],
                )
                nc.gpsimd.collective_compute(
                    "AllGather", mybir.AluOpType.bypass,
                    replica_groups=groups, ins=[s_ci.ap()], outs=[s_if.ap()],
                )
                nc.scalar.dma_start(s_ifl.ap(), s_if.ap())

            # ---- phase 2: gathers on both engines ----
            with tc.tile_pool(name="gpool", bufs=1) as gpool:
                table_t = gpool.tile([128, SLICE], f32)
                # user slice g//4 is contiguous in the allgathered table
                s_ufl_r = s_ufl.ap().rearrange("(s x) one -> s (x one)", s=2)
                for g in range(8):
                    nc.scalar.dma_start(
                        table_t[16 * g:16 * g + 1, :],
                        s_ufl_r[g // 4:g // 4 + 1, :],
                    )

                gu_t = gpool.tile([128, AQG], f32)
                gi2a = gpool.tile([8, AQG], f32)
                gu2b = gpool.tile([NBI, BCH], f32)
                gi2b = gpool.tile([NBI, BCH], f32)

                _q = [0]

                def ind(out_tile, row, table_ap, idx_t, col0, ncols):
                    inst = nc.gpsimd.indirect_dma_start(
                        out=out_tile[row:row + 1, :].rearrange(
                            "one (c x) -> one c x", x=1),
                        out_offset=None,
                        in_=table_ap,
                        in_offset=IndirectOffsetOnAxis(
                            ap=idx_t[:, col0:col0 + ncols], axis=0
                        ),
                    )
                    qn = _q[0] % 4
                    _q[0] += 1
                    if qn:
                        inst.ins.queue = f"qPoolDynamic{qn}"
                    return inst

                def apg(ci):
                    nc.gpsimd.ap_gather(
                        out_ap=gu_t[:, ci * GCH:(ci + 1) * GCH],
                        in_ap=table_t[:],
                        idxs_ap=idxau_t[:, ci * (GCH // 16):(ci + 1) * (GCH // 16)],
                        channels=128,
                        num_elems=SLICE,
                        d=1,
                        num_idxs=GCH,
                    )

                # user-side legs first (user AG lands early); slot the
                # first ap chunks before any item-dependent instruction so
                # the Pool never stalls with ready ap work queued behind a
                # not-yet-ready indirect
                for m in range(NBI):
                    ind(gu2b, m, s_ufl.ap(), idxdu_t, m * COLS_B, COLS_B)
                apg(0)
                apg(1)
                for x in range(6):
                    ind(gi2a, x, s_ifl.ap(), idxda_t, x * COLS_A, COLS_A)
                    apg(x + 2)
                ind(gi2a, 6, s_ifl.ap(), idxda_t, 6 * COLS_A, COLS_A)
                ind(gi2a, 7, s_ifl.ap(), idxda_t, 7 * COLS_A, COLS_A)
                for m in range(NBI):
                    ind(gi2b, m, s_ifl.ap(), idxdi_t, m * COLS_B, COLS_B)
                # outB join
                nc.vector.tensor_add(gu2b[:], gu2b[:], gi2b[:])
                nc.sync.dma_start(outB.ap(), gu2b[:])

                # outA join: realign user ap values, add item, store
                ga_u = gpool.tile([8, AQG], f32)
                nc.scalar.dma_start(ga_u[:], gu_t[0:128:16, :])
                nc.vector.tensor_add(ga_u[:], ga_u[:], gi2a[:])
                nc.sync.dma_start(outA.ap(), ga_u[:])

    nc.compile()
    _CACHE["nc"] = nc
    return nc


def _wrap_dma_idx(flat, n_instr, cols):
    """Spray order: idx[p, q*cols + c] = flat[q*(cols*128) + c*128 + p]."""
    seg = flat.reshape(n_instr, cols, 128)
    return np.ascontiguousarray(
        seg.transpose(2, 0, 1).reshape(128, n_instr * cols))


def _wrap_ap_idx(flat):
    """flat [8 streams, AQG] -> [128, 8*GCH/16] int16 with slot
    t = c*GCH + s*16 + p at idx[16g+p, c*(GCH//16)+s]."""
    A = flat.reshape(8, 8, GCH // 16, 16)          # [g, c, s, p]
    return np.ascontiguousarray(
        A.transpose(0, 3, 1, 2).reshape(128, 8 * (GCH // 16)))


def _make_in_maps(inputs):
    z_user = np.ascontiguousarray(np.asarray(inputs["z_user"], dtype=np.float32))
    z_item = np.ascontiguousarray(np.asarray(inputs["z_item"], dtype=np.float32))
    src = np.asarray(inputs["edge_src"]).astype(np.int64)
    dst = np.asarray(inputs["edge_dst"]).astype(np.int64)
    w_user = np.asarray(inputs["w_user"], dtype=np.float32)
    w_item = np.asarray(inputs["w_item"], dtype=np.float32)
    b_user = np.asarray(inputs["b_user"], dtype=np.float32).reshape(HALF, 1)
    b_item = np.asarray(inputs["b_item"], dtype=np.float32).reshape(HALF, 1)
    w_out = np.asarray(inputs["w_out"], dtype=np.float32)
    b_out = np.asarray(inputs["b_out"], dtype=np.float32).reshape(1, 1)
    wo_u = w_out[0, :HALF].reshape(HALF, 1).copy()
    wo_i = w_out[0, HALF:].reshape(HALF, 1).copy()

    # global offsets into the allgathered tables
    gu32 = ((src // U_SH) * U_PAD + (src % U_SH)).astype(np.int64)
    gi32 = ((dst // I_SH) * I_PAD + (dst % I_SH)).astype(np.int64)
    # user slice-local offsets (ap leg)
    lu16 = (((src // U_SH) % 4) * U_PAD + (src % U_SH)).astype(np.int64)
    subin = (src // 25000).astype(np.int64)        # 0/1 src half

    in_maps = []
    placements = []
    for k in range(N_CORES):
        lo, hi = k * E_SH, (k + 1) * E_SH
        sb = subin[lo:hi]
        order = np.argsort(sb, kind="stable")
        n0 = int(np.sum(sb == 0))
        sel = order + lo                            # edges, su-sorted

        lu_ap = np.zeros((8, AQG), dtype=np.int64)   # ap leg user idx
        gi_ap = np.zeros((8, AQG), dtype=np.int64)   # item idx for A-range
        gu_b = np.zeros(NB, dtype=np.int64)
        gi_b = np.zeros(NB, dtype=np.int64)
        slots = np.empty(E_SH, dtype=np.int64)       # unified slot id

        bpos = 0
        for b, (s0, s1) in enumerate(((0, n0), (n0, E_SH))):
            eb = sel[s0:s1]
            nb_ = len(eb)
            na = min(nb_, 4 * AQG)
            assert nb_ - na <= NB - bpos, f"B overflow core {k}"
            ea = eb[:na]
            g = b * 4 + np.arange(na) // AQG
            t = np.arange(na) % AQG
            lu_ap[g, t] = lu16[ea]
            gi_ap[g, t] = gi32[ea]
            slots[s0:s0 + na] = g * AQG + t
            ebb = eb[na:]
            j = bpos + np.arange(len(ebb))
            gu_b[j] = gu32[ebb]
            gi_b[j] = gi32[ebb]
            slots[s0 + na:s1] = 8 * AQG + j
            bpos += len(ebb)

        idxau_k = _wrap_ap_idx(lu_ap).astype(np.int16)
        idxda_k = _wrap_dma_idx(gi_ap.reshape(-1), 8, COLS_A).astype(np.int32)
        idxdu_k = _wrap_dma_idx(gu_b, NBI, COLS_B).astype(np.int32)
        idxdi_k = _wrap_dma_idx(gi_b, NBI, COLS_B).astype(np.int32)

        zu_k = np.zeros((U_PAD, H), dtype=np.float32)
        zu_k[:U_SH] = z_user[k * U_SH:(k + 1) * U_SH]
        zi_k = np.zeros((I_PAD, H), dtype=np.float32)
        zi_k[:I_SH] = z_item[k * I_SH:(k + 1) * I_SH]

        placements.append((sel, slots))
        in_maps.append({
            "zu": zu_k.reshape(128, UB * H),
            "zi": zi_k.reshape(128, IB * H),
            "w_user": w_user,
            "w_item": w_item,
            "wo_u": wo_u,
            "wo_i": wo_i,
            "b_user": b_user,
            "b_item": b_item,
            "b_out": b_out,
            "idxau": idxau_k,
            "idxda": idxda_k,
            "idxdu": idxdu_k,
            "idxdi": idxdi_k,
        })
    return in_maps, placements


def _run(inputs, trace=False):
    from concourse.bass_utils import run_bass_kernel_spmd

    nc = _build()
    in_maps, placements = _make_in_maps(inputs)
    res = run_bass_kernel_spmd(
        nc, in_maps, core_ids=list(range(N_CORES)), trace=trace
    )
    full = np.empty(E, dtype=np.float32)
    for k in range(N_CORES):
        outa = np.asarray(res.results[k]["outA"]).reshape(-1)
        outb = np.asarray(res.results[k]["outB"]).reshape(-1)
        vals = np.concatenate([outa, outb])
        sel, slots = placements[k]
        full[sel] = vals[slots]
    return full.reshape(E, 1), res


def kernel(**inputs):
    full, _ = _run(inputs, trace=False)
    return full
